# revision 1
# baseline (speedup 1.0000x reference)
"""Trainium2 Bass kernel for AdjacencyErrorAwareLoss.

Math (reference):
    A_fid = (d_hw == 1) * max(1 - d_error, 0)                    [128,128]
    scores[b,e] = P[b,i_e,:] @ A_fid @ P[b,j_e,:]                [B,E]
    loss = -mean_b( sum_e(w*scores) / max(sum_e w, 1e-8) )

Key transformation: scores[b,e] = S_b[i_e, j_e] where S_b = P_b @ A @ P_b^T.
Per sample: two 128^3 matmuls build S_b, then a weighted gather of E=4096
scalars from the 128x128 score matrix.

Distribution: data-parallel over B=64: 8 NeuronCores x 8 samples. On each
core, sample c is handled by GPSIMD core c (partitions 16c..16c+16).

Gather strategy (ap_gather: 8 GPSIMD cores, each processing its 16
partitions with a shared per-core index list, wrapped (s p) across the
core's partitions; measured cost is linear at ~26 ns/index -- the Q7
read-command latency, ReadOverlap=0 on TRN2 -- so the 4096 indices per
core cost ~105 us and dominate the kernel):
  - partition p = 16c+q holds a masked 16384-entry table:
    table2[p, i*128+j] = S_c[i, j] if i//8 == q else 0, so a single
    shared index idx = i*128+j returns the right value on exactly one
    partition of the group and zero on the other 15 -- no separate
    selector-mask gather or mask multiply is needed.
  - the table is built by zeroing once (invariant background), then 16
    per-q stripe DMAs from a DRAM bounce of the S matrices (SBUF DMAs
    cannot collapse partitions or vary free offsets per partition).
  - all 4096 indices go in ONE ap_gather instruction (each extra
    instruction costs ~2-7 us of launch + index-preamble; back-to-back
    5/3/2/1 chunks measured 145.7/138.1/136.6/~130 us).
  - w is shipped host-permuted into gather-column order and replicated
    16x across each partition group (16 row DMAs), so the weighted
    reduction is one contiguous full-width scalar_tensor_tensor with a
    fused per-partition accumulator; masked zeros on the 15 non-owner
    partitions contribute nothing, and one block-ones matmul collapses
    the per-partition partials per sample in the deferred tail.
  - the repeat loop is unrolled x8 per For_i trip (For_i inserts an
    all-engine barrier per trip) with double-buffered load/matmul/bounce
    tiles, so iteration i+1's prologue can overlap iteration i's
    gathers. Iteration i's tail matmuls are DEFERRED until after
    iteration i+1's matmuls are emitted, so their part1 wait never
    head-of-line-blocks the PE queue; early loads go on the ACT HWDGE
    queue and the gather-critical stripes + tail DMAs on SP, so no
    late-waiting DMA stalls next-iteration loads (HWDGE queues are FIFO
    with sequencer-level head-of-line blocking). Steady state ~130-150
    us/iter depending on measurement window, vs ~184 us un-pipelined.

Approaches measured or priced out (see memory notes): indirect_dma_start
supports only one offset per partition per instruction (row gather, 128
descriptors max); dma_gather addresses HBM at idx*256B granularity (no
scalar gather) and 256B rows for both Pi/Vj would be DMA-bound at ~94
us/NC; scatter_add/local_scatter have the same per-index read-command
floor or wrong semantics; a bf16 paired-entry (d=2) table with
double-buffering measured ~26 us SLOWER back-to-back despite halved
table bytes; DVE-only tails via DRAM-bounce transposes blocked the
stripe DMAs and measured worse.
"""

import numpy as np

B, NL, NP, E = 64, 128, 128, 4096
N_CORES = 8
BPC = B // N_CORES  # samples per NeuronCore


def _patch_tile_drain():
    """This toolchain's walrus rejects >1 sem wait on a Drain; split the
    kernel-tail drain into one drain per pending semaphore."""
    import concourse.tile as tile
    from concourse.vector_clock import ScopedClock, VectorClock

    def _drain_and_barrier_split(self, tick_clock, wait_clock):
        nc = self.nc
        gc = tick_clock.global_clock  # VectorClock
        n = len(gc)
        for p in [i for i in range(n) if gc[i] > 0]:
            vec = VectorClock([gc[i] if i == p else 0 for i in range(n)])
            drain_inst = nc.sync.drain()
            wait_clock.add_sem_waits(drain_inst.ins, ScopedClock({None: vec}))
        nc.all_engine_barrier()
        assert self.sems is not None
        popped = nc._tile_sem_poison_stack.pop()
        assert popped is self._sem_poison
        nc.clear_and_free_semaphores(list(self.sems.allocated().values()))
        nc.all_engine_barrier()

    tile.TileContext._drain_and_barrier = _drain_and_barrier_split


def _split_multi_waits(nc, mybir):
    """Walrus codegen accepts at most one sem wait per instruction ("Too
    many sync wait commands"). Hoist extra waits onto preceding same-engine
    NoOps (engines execute in order, so this blocks equivalently)."""
    k = 0
    for f in nc.m.functions:
        for bb in f.blocks:
            insts = list(bb.instructions)
            out = []
            changed = False
            for ins in insts:
                si = ins.sync_info
                waits = list(si.on_wait) if si is not None and si.on_wait else []
                if len(waits) > 1:
                    changed = True
                    for w in waits[:-1]:
                        nop = mybir.InstNoOp(name=f"xw-{k}", ins=[], outs=[])
                        k += 1
                        nop.engine = ins.engine
                        nop.sync_info = mybir.SyncInfo(on_wait=[w], on_update=[])
                        nc.register_instruction(nop)
                        out.append(nop)
                    ins.sync_info = mybir.SyncInfo(
                        on_wait=[waits[-1]], on_update=list(si.on_update or [])
                    )
                out.append(ins)
            if changed:
                bb.instructions = out


def build_nc(repeat: int = 1, stage: str = "full"):
    """Build the Bass module (single-core SPMD program, run on 8 cores).

    repeat > 1 wraps the body in a hardware loop for timing runs.
    stage in ("loads", "mm", "gather", "full") truncates the body for
    cost bisection.
    """
    import concourse.bass as bass
    import concourse.mybir as mybir
    import concourse.tile as tile
    from concourse import library_config

    _patch_tile_drain()

    AL = mybir.AluOpType
    f32 = mybir.dt.float32
    i32 = mybir.dt.int32
    i16 = mybir.dt.int16

    nc = bass.Bass(detect_race_conditions=False)

    p_d = nc.dram_tensor("p", [BPC, NL, NP], f32, kind="ExternalInput")
    ep_d = nc.dram_tensor("ep", [BPC, E, 4], i32, kind="ExternalInput")
    w_d = nc.dram_tensor("w", [BPC, E], f32, kind="ExternalInput")
    derr_d = nc.dram_tensor("derr", [NP, NP], f32, kind="ExternalInput")
    dhw_d = nc.dram_tensor("dhw", [NP, NP], i32, kind="ExternalInput")
    out_d = nc.dram_tensor("out", [1, 1], f32, kind="ExternalOutput")

    # NEFF-embedded constants
    blockones_np = np.zeros((128, BPC), dtype=np.float32)
    for c in range(BPC):
        blockones_np[16 * c:16 * (c + 1), c] = 1.0
    blockones_d = nc.inline_tensor(blockones_np, name="blockones")
    ones_d = nc.inline_tensor(np.ones((128, 1), dtype=np.float32), name="ones128")
    ident_d = nc.inline_tensor(np.eye(128, dtype=np.float32), name="ident128")


    # single gather instruction: with the deferred tail the reduction no
    # longer chains to chunk boundaries, so every removed ap_gather
    # instruction saves its ~2-7 us launch/index-preamble overhead
    # (back-to-back 5/3/2/1 chunks: 145.7/138.1/136.6/-6.9 us)
    CHUNKS = [(0, 8)]

    with tile.TileContext(nc) as tc:
        with (
            tc.tile_pool(name="persist", bufs=1) as persist,
            tc.tile_pool(name="pp", bufs=2, space="PSUM") as pp,
            tc.tile_pool(name="vall", bufs=1, space="PSUM") as vallp,
            tc.tile_pool(name="pred", bufs=2, space="PSUM") as pred,
            tc.tile_pool(name="sdram", bufs=2, space="DRAM") as sdram,
        ):
            nc.gpsimd.load_library(library_config.ap_gather)

            # ---- persistent tiles (shared across pipeline stages)
            blockones = persist.tile([128, BPC], f32)
            ones128 = persist.tile([128, 1], f32)
            ident = persist.tile([128, 128], f32)
            nc.sync.dma_start(blockones[:], blockones_d[:])
            nc.sync.dma_start(ones128[:], ones_d[:])
            nc.sync.dma_start(ident[:], ident_d[:])

            derr = persist.tile([128, 128], f32)
            dhw = persist.tile([128, 128], i32)
            nc.sync.dma_start(derr[:], derr_d[:])
            nc.sync.dma_start(dhw[:], dhw_d[:])

            # masked gather table: [p, i*128+j] = S[i,j] if i//8 == p%16 else 0
            table2 = persist.tile([128, 16384], f32)
            afid = persist.tile([128, 128], f32)
            scr_sh = persist.tile([128, E], f32, name="scrsh")
            relu_e = persist.tile([128, 128], f32)
            mask_e = persist.tile([128, 128], f32)
            # one tile per gather chunk so chunk i+1's gather write cannot
            # false-serialize against chunk i's reduction reads
            val2s = [persist.tile([128, (b - a) * 512, ], f32, name=f"val2_{i}")
                     for i, (a, b) in enumerate(CHUNKS)]

            # double-buffered tiles: iteration i+1's loads/matmuls/bounce
            # overlap iteration i's gathers in the repeat loop
            NSET = 2
            sets = []
            for s in range(NSET):
                bs = {}
                bs["pall"] = persist.tile([128, BPC, 128], f32, name=f"pall{s}")
                bs["pt_all"] = persist.tile([128, BPC, 128], f32, name=f"pt{s}")
                bs["v_sb"] = persist.tile([128, BPC, 128], f32, name=f"v{s}")
                bs["s_all"] = persist.tile([128, BPC, 128], f32, name=f"s{s}")
                bs["epi"] = persist.tile([128, 256, 4], i32, name=f"epi{s}")
                bs["idx16"] = persist.tile([128, 256], i16, name=f"idx{s}")
                bs["t1"] = persist.tile([128, 256], i32, name=f"t1{s}")
                bs["w_nat"] = persist.tile([BPC, E], f32, name=f"w{s}")
                bs["w_g"] = persist.tile([128, E], f32, name=f"wg{s}")
                bs["part1"] = persist.tile([128, 1], f32, name=f"pp1{s}")
                bs["zaccs"] = persist.tile([128, 8], f32, name=f"za{s}")
                bs["ws8"] = persist.tile([BPC, 1], f32, name=f"ws{s}")
                bs["zdiv"] = persist.tile([BPC, 1], f32, name=f"zd{s}")
                bs["res"] = persist.tile([1, 1], f32, name=f"res{s}")
                sets.append(bs)

            # zero the masked table once (stripes are rewritten in place every
            # iteration; the zero background is invariant)
            nc.vector.memset(table2[:, 0:8192], 0.0)
            nc.scalar.memzero(table2[:, 8192:16384])

            def body(_it=0):
                bs = sets[_it % NSET]
                pall = bs["pall"]
                pt_all = bs["pt_all"]
                v_sb = bs["v_sb"]
                s_all = bs["s_all"]
                epi = bs["epi"]
                idx16 = bs["idx16"]
                t1 = bs["t1"]
                w_nat = bs["w_nat"]
                w_g = bs["w_g"]
                scr2 = scr_sh
                zaccs = bs["zaccs"]
                part1 = bs["part1"]
                ws8 = bs["ws8"]
                res = bs["res"]
                # ---- P first: it feeds the transpose->mm1->mm2 PE chain,
                # which is the longest pre-gather dependency path
                p_src = bass.AP(
                    tensor=p_d, offset=0,
                    ap=[[128, 128], [NL * NP, BPC], [1, 128]],
                )
                nc.scalar.dma_start(pall[:], p_src)

                # ---- A_fid = (dhw == 1) * relu(1 - derr)
                nc.scalar.activation(
                    relu_e[:], derr[:],
                    mybir.ActivationFunctionType.Relu, bias=1.0, scale=-1.0,
                )
                nc.vector.tensor_scalar(
                    out=mask_e[:], in0=dhw[:], scalar1=1, scalar2=None,
                    op0=AL.is_equal,
                )
                nc.vector.tensor_tensor(
                    out=afid[:], in0=relu_e[:], in1=mask_e[:], op=AL.mult,
                )

                # ---- edge pairs, contiguous: partition 16c+r holds edges
                # [256r, 256r+256) of sample c; gather position k of sample c
                # is edge 256*(k%16) + k//16
                ep_src = bass.AP(
                    tensor=ep_d, offset=0,
                    ap=[[1024, 128], [4, 256], [1, 4]],
                )
                nc.scalar.dma_start(epi[:], ep_src)
                # idx = i*128 + j  (i = int32 word 0, j = word 2)
                nc.vector.scalar_tensor_tensor(
                    out=t1[:], in0=epi[:, :, 0], scalar=128,
                    in1=epi[:, :, 2], op0=AL.mult, op1=AL.add,
                )
                nc.vector.tensor_copy(idx16[:], t1[:])

                # ---- w natural (permuted) layout; wsum computed early
                nc.scalar.dma_start(w_nat[:], w_d[:])
                # w_g[16c+q, :] = w[c, :]: 16x row replication so the
                # weighted reduction is one full-width stt over the masked
                # values (w is already host-permuted to gather-column order)
                for q in range(16):
                    nc.scalar.dma_start(w_g[q::16, :], w_d[:])
                nc.vector.tensor_reduce(
                    out=ws8[:], in_=w_nat[:], axis=mybir.AxisListType.X, op=AL.add,
                )
                nc.vector.tensor_scalar(
                    out=ws8[:], in0=ws8[:], scalar1=1e-8, scalar2=None, op0=AL.max,
                )
                nc.vector.reciprocal(ws8[:], ws8[:])

                if stage == "loads":
                    nc.vector.memset(res[:], 0.0)
                    nc.sync.dma_start(out_d[:], res[:])
                    return

                # ---- per-sample transposes, then batched V = mm(A, P^T)
                for c in range(BPC):
                    pt_ps = pp.tile([128, 128], f32, tag="ptps")
                    nc.tensor.transpose(pt_ps[:], pall[:, c, :], ident[:])
                    nc.scalar.copy(pt_all[:, c, :], pt_ps[:])

                v_ps = vallp.tile([128, BPC, 128], f32)
                nc.tensor.matmul(
                    v_ps[:].rearrange("p a b -> p (a b)")[:, 0:512],
                    lhsT=afid[:],
                    rhs=pt_all[:].rearrange("p a b -> p (a b)")[:, 0:512],
                    start=True, stop=True,
                )
                nc.tensor.matmul(
                    v_ps[:].rearrange("p a b -> p (a b)")[:, 512:1024],
                    lhsT=afid[:],
                    rhs=pt_all[:].rearrange("p a b -> p (a b)")[:, 512:1024],
                    start=True, stop=True,
                )
                nc.vector.tensor_copy(v_sb[:], v_ps[:])

                # ---- S_c = V_c^T(as lhsT) @ P_c^T ; copy to s_all; bounce
                # (per-sample DRAM writes overlap the remaining matmuls)
                s_dr = sdram.tile([128, BPC, 128], f32, tag="sdram")
                for c in range(BPC):
                    s_ps = pp.tile([128, 128], f32, tag="sps")
                    nc.tensor.matmul(
                        s_ps[:], lhsT=v_sb[:, c, :], rhs=pt_all[:, c, :],
                        start=True, stop=True,
                    )
                    nc.scalar.copy(s_all[:, c, :], s_ps[:])
                    nc.scalar.dma_start(s_dr[:, c, :], s_all[:, c, :])
                # 16 stripe reads: table2[{16c+q}, 1024q : 1024q+1024] =
                #   S_c rows [8q, 8q+8)
                for q in range(16):
                    dst = table2[q::16, 1024 * q:1024 * (q + 1)]
                    nc.sync.dma_start(
                        dst,
                        s_dr[:].rearrange("l c x -> c l x")[:, 8 * q:8 * q + 8, :],
                    )

                if stage == "mm":
                    nc.vector.memset(res[:], 0.0)
                    nc.sync.dma_start(out_d[:], res[:])
                    return

                # ---- gather + reduction, chunked: the PE/DVE reduction of
                # chunk ch overlaps the GPSIMD gather of chunk ch+1
                for ch, (a, b) in enumerate(CHUNKS):
                    ss = slice(32 * a, 32 * b)
                    val2 = val2s[ch]
                    nc.gpsimd.ap_gather(
                        out_ap=val2[:].unsqueeze(2),
                        in_ap=table2[:].unsqueeze(2),
                        idxs_ap=idx16[:, ss],
                        channels=128, num_elems=16384, d=1,
                        num_idxs=(b - a) * 512,
                    )
                    if stage == "gonly":
                        continue
                    # weighted partial sums: one full-width stt per chunk;
                    # masked zeros on 15/16 partitions contribute nothing, so
                    # per-partition accumulators sum to the per-sample total.
                    # gather column k is edge 256*(k%16) + k//16, so in1 views
                    # w_g as [p, hi, lo] with lo innermost at stride 256
                    ncol = (b - a) * 512
                    nc.vector.scalar_tensor_tensor(
                        out=scr2[:, 0:ncol],
                        in0=val2[:],
                        scalar=0.0, in1=w_g[:, 512 * a:512 * b],
                        op0=AL.add, op1=AL.mult,
                        accum_out=zaccs[:, ch:ch + 1],
                    )

                if stage == "gonly":
                    nc.vector.tensor_copy(res[:], val2s[0][0:1, 0:1])
                    nc.sync.dma_start(out_d[:], res[:])
                    return

                nc.vector.tensor_reduce(
                    out=part1[:], in_=zaccs[:, 0:len(CHUNKS)],
                    axis=mybir.AxisListType.X, op=AL.add,
                )

                if stage == "gather":
                    nc.vector.memset(res[:], 0.0)
                    nc.sync.dma_start(out_d[:], res[:])
                    return

            def body_tail(_it=0):
                # finals for iteration _it, emitted AFTER iteration _it+1's
                # matmuls so the part1 wait never head-of-line-blocks the PE
                # queue: collapse 16-partition groups, divide, sum
                bs = sets[_it % NSET]
                y_ps = pred.tile([BPC, 1], f32, tag="y")
                nc.tensor.matmul(
                    y_ps[:], lhsT=blockones[:], rhs=bs["part1"][:],
                    start=True, stop=True,
                )
                nc.vector.tensor_tensor(
                    out=bs["zdiv"][:], in0=y_ps[:], in1=bs["ws8"][:],
                    op=AL.mult,
                )
                zz_ps = pred.tile([1, 1], f32, tag="y")
                nc.tensor.matmul(
                    zz_ps[:], lhsT=bs["zdiv"][:], rhs=ones128[0:BPC, :],
                    start=True, stop=True,
                )
                nc.vector.tensor_copy(bs["res"][:], zz_ps[:])
                nc.vector.tensor_scalar_mul(bs["res"][:], bs["res"][:],
                                            -1.0 / B)
                nc.sync.dma_start(out_d[:], bs["res"][:])

            def emit_seq(n):
                # main(0), main(1), tail(0), main(2), tail(1), ... tail(n-1)
                for it in range(n):
                    body(it)
                    if stage == "full" and it > 0:
                        body_tail(it - 1)
                if stage == "full":
                    body_tail(n - 1)

            if repeat == 1:
                body()
                if stage == "full":
                    body_tail(0)
            elif repeat <= 8:
                # flat unroll (no HW loop) -- lets TimelineSim run it
                emit_seq(repeat)
            else:
                # For_i inserts an all-engine barrier per trip; unroll x8
                # bodies per trip (alternating buffer sets) so the barrier
                # and pipeline ramp amortize over 8 iterations
                UN = 8
                with tc.For_i(0, repeat // UN, 1):
                    emit_seq(UN)
                if repeat % UN:
                    emit_seq(repeat % UN)


    _split_multi_waits(nc, mybir)
    # Populate .instr bytes for extended-inst InstISA subclasses (ap_gather);
    # without this the NEFF compiler sees empty .instr -> "ISA wrong length".
    mybir.codegen_inst_isa_subclasses(nc)
    return nc


def _shard_inputs(P, d_error, edge_weights, d_hw, edge_pairs):
    ep32 = edge_pairs.astype(np.int64, copy=False).view(np.int32).reshape(B, E, 4)
    derr = np.ascontiguousarray(d_error, dtype=np.float32)
    dhw = np.ascontiguousarray(d_hw, dtype=np.int32)
    # gather-column order: column k of the gathered values is edge
    # 256*(k%16) + k//16, so ship w permuted to make the device-side
    # weighted reduction a contiguous packed view (sums are order-invariant)
    k = np.arange(E)
    perm = 256 * (k % 16) + k // 16
    w_perm = np.ascontiguousarray(edge_weights[:, perm], dtype=np.float32)
    in_maps = []
    for core in range(N_CORES):
        s = slice(BPC * core, BPC * (core + 1))
        in_maps.append({
            "p": np.ascontiguousarray(P[s], dtype=np.float32),
            "ep": np.ascontiguousarray(ep32[s]),
            "w": w_perm[s],
            "derr": derr,
            "dhw": dhw,
        })
    return in_maps


def kernel(P, d_error, edge_weights, d_hw, edge_pairs):
    from concourse.bass_utils import run_bass_kernel_spmd

    nc = build_nc()
    in_maps = _shard_inputs(P, d_error, edge_weights, d_hw, edge_pairs)
    res = run_bass_kernel_spmd(nc, in_maps, core_ids=list(range(N_CORES)))
    total = np.float32(0.0)
    for r in res.results:
        total += np.float32(r["out"][0, 0])
    return np.float32(total)



# revision 23
# speedup vs baseline: 1.7559x; 1.7559x over previous
"""Trainium2 Bass kernel for AdjacencyErrorAwareLoss — hybrid gather v2.

Math (reference):
    A_fid = (d_hw == 1) * max(1 - d_error, 0)                    [128,128]
    scores[b,e] = P[b,i_e,:] @ A_fid @ P[b,j_e,:]                [B,E]
    loss = -mean_b( sum_e(w*scores) / max(sum_e w, 1e-8) )

Key identity: scores[b,e] = S_b[i_e, j_e] with S_b = P_b @ A @ P_b^T, and
only the weighted SUM of scores is needed:
    sum_e w_e S_b[i_e,j_e] = <W_b, S_b>_F,  W_b = sum_e w_e (e_i x e_j).

v2 splits each sample's 4096 edges between two engines:
  - N_G edges/sample go through the v1 ap_gather path (8 GPSIMD cores, one
    per sample, masked f32 table of S values, ~26 ns/index serial/core).
  - The other 4096-N_G edges go through a one-hot matmul scatter:
      DVE builds bf16 one-hots OH[e,v] = (idx_e == v) at ~0.35 ns/FD-elem
      (2x mode; idx broadcast on the MIDDLE axis keeps step-1 inner runs),
      folds w into the I side (~0.64 ns/FD), then PE accumulates
      W_b = sum_chunks OHW_I_c^T @ OH_J_c into PSUM (128-edge chunks).
      Measured: strided lhsT is free, strided rhs costs ~3x (214 vs <90
      ns/MM), so the J one-hot is re-laid chunk-contiguous by the
      otherwise-idle ACT engine (~0.22 ns/FD) and used as rhs; the I
      one-hot stays in its DVE-native [p, i, slot] layout as lhsT.
      Per sample: <W_b(PSUM), S_b(SBUF)> via one fused tensor_tensor_reduce.
  - ap_gather table must be f32: ap_gather requires d*dtype_size % 4 == 0.

Distribution: data-parallel over B=64: 8 NeuronCores x 8 samples.
Host pre-packs index/weight tiles in the exact SBUF layouts (pure
data-independent permutations/dtype casts).
"""

import numpy as np

B, NL, NP, E = 64, 128, 128, 4096
N_CORES = 8
BPC = B // N_CORES  # samples per NeuronCore

# hybrid split: per sample, N_G edges on the GPSIMD gather path, the rest
# on the DVE/PE one-hot path in M_CH chunks of 128 edges
N_G = 1280
M_CH = (E - N_G) // 128      # 22
N_DVE = E - N_G              # 2816
QS = 1                       # samples per one-hot batch op
NQ = BPC // QS               # batches
SLOTS = QS * M_CH            # 44 chunk-slots per batch
FDQ = 128 * SLOTS            # one-hot free dim per batch op


def _patch_tile_drain():
    """This toolchain's walrus rejects >1 sem wait on a Drain; split the
    kernel-tail drain into one drain per pending semaphore."""
    import concourse.tile as tile
    from concourse.vector_clock import ScopedClock, VectorClock

    def _drain_and_barrier_split(self, tick_clock, wait_clock):
        nc = self.nc
        gc = tick_clock.global_clock  # VectorClock
        n = len(gc)
        for p in [i for i in range(n) if gc[i] > 0]:
            vec = VectorClock([gc[i] if i == p else 0 for i in range(n)])
            drain_inst = nc.sync.drain()
            wait_clock.add_sem_waits(drain_inst.ins, ScopedClock({None: vec}))
        nc.all_engine_barrier()
        assert self.sems is not None
        popped = nc._tile_sem_poison_stack.pop()
        assert popped is self._sem_poison
        nc.clear_and_free_semaphores(list(self.sems.allocated().values()))
        nc.all_engine_barrier()

    tile.TileContext._drain_and_barrier = _drain_and_barrier_split


def _split_multi_waits(nc, mybir):
    """Walrus codegen accepts at most one sem wait per instruction. Hoist
    extra waits onto preceding same-engine NoOps."""
    k = 0
    for f in nc.m.functions:
        for bb in f.blocks:
            insts = list(bb.instructions)
            out = []
            changed = False
            for ins in insts:
                si = ins.sync_info
                waits = list(si.on_wait) if si is not None and si.on_wait else []
                if len(waits) > 1:
                    changed = True
                    for w in waits[:-1]:
                        nop = mybir.InstNoOp(name=f"xw-{k}", ins=[], outs=[])
                        k += 1
                        nop.engine = ins.engine
                        nop.sync_info = mybir.SyncInfo(on_wait=[w], on_update=[])
                        nc.register_instruction(nop)
                        out.append(nop)
                    ins.sync_info = mybir.SyncInfo(
                        on_wait=[waits[-1]], on_update=list(si.on_update or [])
                    )
                out.append(ins)
            if changed:
                bb.instructions = out


def build_nc(repeat: int = 1, stage: str = "full"):
    """Build the Bass module (single-core SPMD program, run on 8 cores).

    stage in ("loads", "sbuild", "gather", "dve", "full") truncates the
    body for cost bisection.
    """
    import concourse.bass as bass
    import concourse.mybir as mybir
    import concourse.tile as tile
    from concourse import library_config

    _patch_tile_drain()

    AL = mybir.AluOpType
    f32 = mybir.dt.float32
    bf16 = mybir.dt.bfloat16
    i16 = mybir.dt.int16

    nc = bass.Bass(detect_race_conditions=False)

    p_d = nc.dram_tensor("p", [BPC, NL, NP], f32, kind="ExternalInput")
    idxg_d = nc.dram_tensor("idxg", [128, N_G // 16], i16, kind="ExternalInput")
    idxi_d = nc.dram_tensor("idxi", [128, QS * M_CH * NQ], i16, kind="ExternalInput")
    idxj_d = nc.dram_tensor("idxj", [128, QS * M_CH * NQ], i16, kind="ExternalInput")
    wdve_d = nc.dram_tensor("wdve", [128, QS * M_CH * NQ], f32, kind="ExternalInput")
    wg_d = nc.dram_tensor("wg", [BPC, N_G], f32, kind="ExternalInput")
    # w reshaped so partition 16c+q holds w[c, 256q:256q+256]
    wnat_d = nc.dram_tensor("wnat", [128, 256], f32, kind="ExternalInput")
    derr_d = nc.dram_tensor("derr", [NP, NP], f32, kind="ExternalInput")
    dhw_d = nc.dram_tensor("dhw", [NP, NP], i32 := mybir.dt.int32, kind="ExternalInput")
    out_d = nc.dram_tensor("out", [1, 1], f32, kind="ExternalOutput")

    # NEFF-embedded constants
    blockones_np = np.zeros((128, BPC), dtype=np.float32)
    for c in range(BPC):
        blockones_np[16 * c:16 * (c + 1), c] = 1.0
    blockones_d = nc.inline_tensor(blockones_np, name="blockones")
    ones_d = nc.inline_tensor(np.ones((128, 1), dtype=np.float32), name="ones128")
    ident_d = nc.inline_tensor(np.eye(128, dtype=np.float32), name="ident128")
    # iota for one-hot builds: value at free pos (v*SLOTS + s) is v
    iota_np = np.tile(np.repeat(np.arange(128, dtype=np.int16), SLOTS), (128, 1))
    iota_d = nc.inline_tensor(iota_np, name="iotaw")

    with tile.TileContext(nc) as tc:
        with (
            tc.tile_pool(name="persist", bufs=1) as persist,
            tc.tile_pool(name="pp", bufs=2, space="PSUM") as pp,
            tc.tile_pool(name="vall", bufs=1, space="PSUM") as vallp,
            tc.tile_pool(name="pred", bufs=1, space="PSUM") as pred,
            tc.tile_pool(name="sdram", bufs=2, space="DRAM") as sdram,
        ):
            nc.gpsimd.load_library(library_config.ap_gather)

            # ---- persistent constants
            blockones = persist.tile([128, BPC], f32)
            ones128 = persist.tile([128, 1], f32)
            ident = persist.tile([128, 128], f32)
            iota = persist.tile([128, FDQ], i16)
            nc.sync.dma_start(blockones[:], blockones_d[:])
            nc.sync.dma_start(ones128[:], ones_d[:])
            nc.sync.dma_start(ident[:], ident_d[:])
            nc.sync.dma_start(iota[:], iota_d[:])

            derr = persist.tile([128, 128], f32)
            dhw = persist.tile([128, 128], i32)
            nc.sync.dma_start(derr[:], derr_d[:])
            nc.sync.dma_start(dhw[:], dhw_d[:])

            # masked gather table: [p, i*128+j] = S_c[i,j] if i//8 == p%16 else 0
            table2 = persist.tile([128, 16384], f32)
            afid = persist.tile([128, 128], f32)
            relu_e = persist.tile([128, 128], f32)
            mask_e = persist.tile([128, 128], f32)
            nc.vector.memset(table2[:, 0:8192], 0.0)
            nc.scalar.memzero(table2[:, 8192:16384])

            # A_fid = (dhw == 1) * relu(1 - derr)   (invariant; hoisted)
            nc.scalar.activation(
                relu_e[:], derr[:],
                mybir.ActivationFunctionType.Relu, bias=1.0, scale=-1.0,
            )
            nc.vector.tensor_scalar(
                out=mask_e[:], in0=dhw[:], scalar1=1, scalar2=None,
                op0=AL.is_equal,
            )
            nc.vector.tensor_tensor(
                out=afid[:], in0=relu_e[:], in1=mask_e[:], op=AL.mult,
            )

            # ---- one-hot working buffers (single/double per role)
            ohI_raw = persist.tile([128, FDQ], bf16, name="ohIraw")
            ohIw = [persist.tile([128, FDQ], bf16, name=f"ohIw{k}") for k in range(2)]
            ohJ_raw = persist.tile([128, FDQ], bf16, name="ohJraw")
            ohJci = [persist.tile([128, FDQ], bf16, name=f"ohJci{k}") for k in range(2)]
            ttr_scr = persist.tile([128, 128], bf16, name="ttrscr")
            w_sb = persist.tile([128, BPC, 128], bf16, name="wsb")

            # ---- per-iteration tiles (double-buffered across repeat iters)
            NSET = 2
            sets = []
            for s in range(NSET):
                bs = {}
                bs["pall"] = persist.tile([128, BPC, 128], f32, name=f"pall{s}")
                bs["pt_all"] = persist.tile([128, BPC, 128], f32, name=f"pt{s}")
                bs["v_sb"] = persist.tile([128, BPC, 128], f32, name=f"v{s}")
                bs["s_all"] = persist.tile([128, BPC, 128], f32, name=f"s{s}")
                bs["idxg"] = persist.tile([128, N_G // 16], i16, name=f"idxg{s}")
                bs["idxi"] = persist.tile([128, NQ * SLOTS], i16, name=f"idxi{s}")
                bs["idxj"] = persist.tile([128, NQ * SLOTS], i16, name=f"idxj{s}")
                bs["wdve32"] = persist.tile([128, NQ * SLOTS], f32, name=f"wd32{s}")
                bs["wdve"] = persist.tile([128, NQ * SLOTS], bf16, name=f"wd{s}")
                bs["w_nat"] = persist.tile([128, 256], f32, name=f"w{s}")
                bs["wspart"] = persist.tile([128, 1], f32, name=f"wsp{s}")
                bs["w_g"] = persist.tile([128, N_G], f32, name=f"wg{s}")
                bs["val2"] = persist.tile([128, N_G], f32, name=f"val2{s}")
                bs["scr"] = persist.tile([128, N_G], f32, name=f"scr{s}")
                bs["zacc"] = persist.tile([128, 1], f32, name=f"za{s}")
                bs["ttrcols"] = persist.tile([128, BPC], f32, name=f"ttrc{s}")
                bs["ws8"] = persist.tile([BPC, 1], f32, name=f"ws{s}")
                bs["ysum"] = persist.tile([BPC, 1], f32, name=f"ys{s}")
                bs["zdiv"] = persist.tile([BPC, 1], f32, name=f"zd{s}")
                bs["res"] = persist.tile([1, 1], f32, name=f"res{s}")
                sets.append(bs)

            def body(_it=0):
                bs = sets[_it % NSET]
                pall, pt_all = bs["pall"], bs["pt_all"]
                v_sb, s_all = bs["v_sb"], bs["s_all"]

                # ---- loads (ACT HWDGE queue for early loads)
                p_src = bass.AP(
                    tensor=p_d, offset=0,
                    ap=[[128, 128], [NL * NP, BPC], [1, 128]],
                )
                nc.scalar.dma_start(pall[:], p_src)
                nc.scalar.dma_start(bs["idxg"][:], idxg_d[:])
                nc.scalar.dma_start(bs["idxi"][:], idxi_d[:])
                nc.scalar.dma_start(bs["idxj"][:], idxj_d[:])
                nc.scalar.dma_start(bs["wdve32"][:], wdve_d[:])
                nc.scalar.dma_start(bs["w_nat"][:], wnat_d[:])
                for q in range(16):
                    nc.scalar.dma_start(bs["w_g"][q::16, :], wg_d[:])
                nc.vector.tensor_copy(bs["wdve"][:], bs["wdve32"][:])

                nc.vector.tensor_reduce(
                    out=bs["wspart"][:], in_=bs["w_nat"][:],
                    axis=mybir.AxisListType.X, op=AL.add,
                )

                if stage == "loads":
                    nc.vector.memset(bs["res"][:], 0.0)
                    nc.sync.dma_start(out_d[:], bs["res"][:])
                    return

                # ---- S_b = P_b @ A @ P_b^T  (f32, feeds table + TTR)
                for c in range(BPC):
                    pt_ps = pp.tile([128, 128], f32, tag="ptps")
                    nc.tensor.transpose(pt_ps[:], pall[:, c, :], ident[:])
                    nc.scalar.copy(pt_all[:, c, :], pt_ps[:])
                v_ps = vallp.tile([128, BPC, 128], f32, tag="vw")
                nc.tensor.matmul(
                    v_ps[:].rearrange("p a b -> p (a b)")[:, 0:512],
                    lhsT=afid[:],
                    rhs=pt_all[:].rearrange("p a b -> p (a b)")[:, 0:512],
                    start=True, stop=True,
                )
                nc.tensor.matmul(
                    v_ps[:].rearrange("p a b -> p (a b)")[:, 512:1024],
                    lhsT=afid[:],
                    rhs=pt_all[:].rearrange("p a b -> p (a b)")[:, 512:1024],
                    start=True, stop=True,
                )
                nc.vector.tensor_copy(v_sb[:], v_ps[:])

                s_dr = sdram.tile([128, BPC, 128], f32, tag="sdram")
                for c in range(BPC):
                    s_ps = pp.tile([128, 128], f32, tag="sps")
                    nc.tensor.matmul(
                        s_ps[:], lhsT=v_sb[:, c, :], rhs=pt_all[:, c, :],
                        start=True, stop=True,
                    )
                    nc.scalar.copy(s_all[:, c, :], s_ps[:])
                    nc.scalar.dma_start(s_dr[:, c, :], s_all[:, c, :])
                # 16 stripe reads: table2[{16c+q}, 1024q:1024q+1024] =
                #   S_c rows [8q, 8q+8)
                for q in range(16):
                    dst = table2[q::16, 1024 * q:1024 * (q + 1)]
                    nc.sync.dma_start(
                        dst,
                        s_dr[:].rearrange("l c x -> c l x")[:, 8 * q:8 * q + 8, :],
                    )

                if stage == "sbuild":
                    nc.vector.memset(bs["res"][:], 0.0)
                    nc.sync.dma_start(out_d[:], bs["res"][:])
                    return

                # ---- gather path: one ap_gather (all 8 cores, sample/core)
                if stage in ("gather", "full"):
                    nc.gpsimd.ap_gather(
                        out_ap=bs["val2"][:].unsqueeze(2),
                        in_ap=table2[:].unsqueeze(2),
                        idxs_ap=bs["idxg"][:],
                        channels=128, num_elems=16384, d=1,
                        num_idxs=N_G,
                    )

                # ---- one-hot scatter path, NQ batches of QS samples.
                # W accumulators reuse the vall PSUM banks (V is fully
                # consumed by the S matmuls above before W writes start).
                if stage in ("dve1", "dve2", "dve", "full"):
                    wps4 = vallp.tile([128, BPC, 128], f32, tag="vw")
                    for qb in range(NQ):
                        sl = slice(qb * SLOTS, (qb + 1) * SLOTS)
                        idxI = bs["idxi"][:, sl].unsqueeze(1).broadcast_to(
                            [128, 128, SLOTS])
                        idxJ = bs["idxj"][:, sl].unsqueeze(1).broadcast_to(
                            [128, 128, SLOTS])
                        wq = bs["wdve"][:, sl].unsqueeze(1).broadcast_to(
                            [128, 128, SLOTS])
                        i3 = iota[:].rearrange("p (v s) -> p v s", v=128, s=SLOTS)
                        oJ = ohJ_raw[:].rearrange("p (v s) -> p v s", v=128, s=SLOTS)
                        oI = ohI_raw[:].rearrange("p (v s) -> p v s", v=128, s=SLOTS)
                        oIw = ohIw[qb % 2][:].rearrange(
                            "p (v s) -> p v s", v=128, s=SLOTS)
                        # J one-hot then ACT re-layout to chunk-contiguous
                        nc.vector.tensor_tensor(
                            out=oJ, in0=idxJ, in1=i3, op=AL.is_equal)
                        nc.scalar.copy(
                            ohJci[qb % 2][:].rearrange(
                                "p (s v) -> p s v", s=SLOTS, v=128),
                            oJ.transpose([0, 2, 1]),
                        )
                        # I one-hot, then fold w (both in [p, v, s] layout)
                        nc.vector.tensor_tensor(
                            out=oI, in0=idxI, in1=i3, op=AL.is_equal)
                        nc.vector.tensor_tensor(
                            out=oIw, in0=oI, in1=wq, op=AL.mult)
                        # PE: accumulate W_b for the QS samples of this batch
                        if stage == "dve1":
                            continue
                        for bl in range(QS):
                            b = qb * QS + bl
                            for c in range(M_CH):
                                s = bl * M_CH + c
                                lhs = ohIw[qb % 2][:].rearrange(
                                    "p (v s) -> p v s", v=128, s=SLOTS)[:, :, s]
                                rhs = ohJci[qb % 2][:].rearrange(
                                    "p (s v) -> p s v", s=SLOTS, v=128)[:, s, :]
                                nc.tensor.matmul(
                                    wps4[:, b, :], lhsT=lhs, rhs=rhs,
                                    start=(c == 0), stop=(c == M_CH - 1),
                                )
                        # TTR per sample: <W_b, S_b> -> ttrcols[:, b]
                        # (W evacuated PSUM->SBUF by ACT first; TTR with a
                        # PSUM operand faults on HW)
                        if stage == "dve2":
                            continue
                        for bl in range(QS):
                            b = qb * QS + bl
                            nc.scalar.copy(w_sb[:, b, :], wps4[:, b, :])
                            nc.vector.tensor_tensor(
                                out=ttr_scr[:],
                                in0=w_sb[:, b, :], in1=s_all[:, b, :],
                                op=AL.mult,
                            )
                            nc.vector.tensor_reduce(
                                out=bs["ttrcols"][:, b:b + 1], in_=ttr_scr[:],
                                axis=mybir.AxisListType.X, op=AL.add,
                            )
                else:
                    nc.vector.memset(bs["ttrcols"][:], 0.0)
                if stage in ("dve1", "dve2"):
                    nc.vector.memset(bs["ttrcols"][:], 0.0)

                # ---- gather-path weighted reduction
                if stage in ("gather", "full"):
                    nc.vector.scalar_tensor_tensor(
                        out=bs["scr"][:],
                        in0=bs["val2"][:],
                        scalar=0.0, in1=bs["w_g"][:],
                        op0=AL.add, op1=AL.mult,
                        accum_out=bs["zacc"][:],
                    )
                else:
                    nc.vector.memset(bs["zacc"][:], 0.0)

            def body_tail(_it=0):
                # deferred finals for iteration _it (emitted after iteration
                # _it+1's matmuls so PE queue isn't head-of-line-blocked)
                bs = sets[_it % NSET]
                yv_ps = pred.tile([BPC, 4], f32, tag="yv")
                # y = gather-part + dve-part per sample, accumulated in PSUM
                nc.tensor.matmul(
                    yv_ps[:, 0:1], lhsT=blockones[:], rhs=bs["zacc"][:],
                    start=True, stop=False,
                )
                nc.tensor.matmul(
                    yv_ps[:, 0:1], lhsT=bs["ttrcols"][:], rhs=ones128[:],
                    start=False, stop=True,
                )
                nc.tensor.matmul(
                    yv_ps[:, 2:3], lhsT=blockones[:], rhs=bs["wspart"][:],
                    start=True, stop=True,
                )
                nc.vector.tensor_scalar(
                    out=bs["ws8"][:], in0=yv_ps[:, 2:3], scalar1=1e-8,
                    scalar2=None, op0=AL.max,
                )
                nc.vector.reciprocal(bs["ws8"][:], bs["ws8"][:])
                nc.vector.tensor_tensor(
                    out=bs["zdiv"][:], in0=yv_ps[:, 0:1], in1=bs["ws8"][:],
                    op=AL.mult,
                )
                zz_ps = pred.tile([1, 1], f32, tag="zz")
                nc.tensor.matmul(
                    zz_ps[:], lhsT=bs["zdiv"][:], rhs=ones128[0:BPC, :],
                    start=True, stop=True,
                )
                nc.vector.tensor_copy(bs["res"][:], zz_ps[:])
                nc.vector.tensor_scalar_mul(bs["res"][:], bs["res"][:],
                                            -1.0 / B)
                nc.sync.dma_start(out_d[:], bs["res"][:])

            full_tail = stage in ("full", "dve", "dve1", "dve2", "gather")

            def emit_seq(n):
                for it in range(n):
                    body(it)
                    if full_tail and it > 0:
                        body_tail(it - 1)
                if full_tail:
                    body_tail(n - 1)

            if repeat == 1:
                body()
                if full_tail:
                    body_tail(0)
            elif repeat <= 8:
                emit_seq(repeat)
            else:
                UN = 4
                with tc.For_i(0, repeat // UN, 1):
                    emit_seq(UN)
                if repeat % UN:
                    emit_seq(repeat % UN)

    _split_multi_waits(nc, mybir)
    mybir.codegen_inst_isa_subclasses(nc)
    return nc


def _shard_inputs(P, d_error, edge_weights, d_hw, edge_pairs):
    ep = np.asarray(edge_pairs)
    i_all = ep[..., 0].astype(np.int16)   # [B, E]
    j_all = ep[..., 1].astype(np.int16)
    w_all = np.asarray(edge_weights, dtype=np.float32)
    derr = np.ascontiguousarray(d_error, dtype=np.float32)
    dhw = np.ascontiguousarray(d_hw, dtype=np.int32)

    # --- gather part: edge(q, t) = 256q + t for t < N_G//16; gather column
    # k (of N_G) on partition-row q = k // (N_G//16)... layout: partition
    # 16c+q holds sample c's indices for edges {256q + t : t in [0, TG)}.
    TG = N_G // 16
    t_idx = np.arange(TG)
    q_idx = np.arange(16)
    gather_edges = (256 * q_idx[:, None] + t_idx[None, :])  # [16, TG]

    # --- dve part: leftover lin index L in [0, N_DVE): q = L // TL,
    # t = L % TL, edge = 256q + TG + t ; slot mapping L = c*128 + p
    TL = 256 - TG
    Lp = np.arange(N_DVE)
    dve_edges = 256 * (Lp // TL) + TG + (Lp % TL)   # [N_DVE] in L order
    # tile [128 p, M_CH c] per sample: value at (p, c) = edge L = c*128 + p
    dve_map = dve_edges.reshape(M_CH, 128).T         # [128, M_CH]

    # gather output column k (of N_G, same on all 16 partitions of a
    # core) uses the index stored at wrapped position (row k%16, col
    # k//16), i.e. edge 256*(k%16) + k//16 with k//16 < TG. w_g pairs w
    # with those columns, replicated to all 16 partition rows of a sample
    # (masked table zeros the 15 non-owner rows) -- v1-validated layout.
    k_idx = np.arange(N_G)
    gcol_edges = 256 * (k_idx % 16) + (k_idx // 16)  # [N_G], t = k//16 < TG

    in_maps = []
    for core in range(N_CORES):
        sl = slice(BPC * core, BPC * (core + 1))
        i_c = i_all[sl]   # [BPC, E] int16
        j_c = j_all[sl]
        w_c = w_all[sl]

        # idxg [128, TG]: partition 16c+q, position t -> combined index of
        # sample c, edge 256q + t
        idxg = np.zeros((128, TG), dtype=np.int16)
        for c in range(BPC):
            for q in range(16):
                e_ids = gather_edges[q]
                idxg[16 * c + q, :] = (
                    i_c[c, e_ids].astype(np.int32) * 128
                    + j_c[c, e_ids].astype(np.int32)
                ).astype(np.int16)

        # dve tiles [128, NQ*SLOTS]: slot s = b*M_CH + c global order:
        # columns grouped per batch qb: for qb, local (bl, c):
        # col = qb*SLOTS + bl*M_CH + c, sample b = qb*QS+bl, edge =
        # dve_map[p, c]
        idxi = np.zeros((128, NQ * SLOTS), dtype=np.int16)
        idxj = np.zeros((128, NQ * SLOTS), dtype=np.int16)
        wdve = np.zeros((128, NQ * SLOTS), dtype=np.float32)
        for b in range(BPC):
            qb, bl = divmod(b, QS)
            cols = qb * SLOTS + bl * M_CH + np.arange(M_CH)
            idxi[:, cols] = i_c[b, dve_map]
            idxj[:, cols] = j_c[b, dve_map]
            wdve[:, cols] = w_c[b, dve_map]

        wg = np.ascontiguousarray(w_c[:, gcol_edges])  # [BPC, N_G]

        in_maps.append({
            "p": np.ascontiguousarray(P[sl], dtype=np.float32),
            "idxg": idxg,
            "idxi": idxi,
            "idxj": idxj,
            "wdve": wdve,
            "wg": wg,
            "wnat": np.ascontiguousarray(w_c.reshape(BPC * 16, 256)),
            "derr": derr,
            "dhw": dhw,
        })
    return in_maps


def kernel(P, d_error, edge_weights, d_hw, edge_pairs):
    from concourse.bass_utils import run_bass_kernel_spmd

    nc = build_nc()
    in_maps = _shard_inputs(P, d_error, edge_weights, d_hw, edge_pairs)
    res = run_bass_kernel_spmd(nc, in_maps, core_ids=list(range(N_CORES)))
    total = np.float32(0.0)
    for r in res.results:
        total += np.float32(r["out"][0, 0])
    return np.float32(total)


# revision 55
# speedup vs baseline: 2.1559x; 1.2278x over previous
"""Trainium2 Bass kernel for AdjacencyErrorAwareLoss — hybrid scatter v2.

Math (reference):
    A_fid = (d_hw == 1) * max(1 - d_error, 0)                    [128,128]
    scores[b,e] = P[b,i_e,:] @ A_fid @ P[b,j_e,:]                [B,E]
    loss = -mean_b( sum_e(w*scores) / max(sum_e w, 1e-8) )

Key identity: scores[b,e] = S_b[i_e, j_e] with S_b = P_b @ A @ P_b^T, and
only the weighted SUM of scores is needed:
    sum_e w_e S_b[i_e,j_e] = <W_b, S_b>_F,  W_b = sum_e w_e (e_i x e_j).

Distribution: data-parallel over B=64: 8 NeuronCores x 8 samples. Inputs
are host-packed into exact SBUF tile layouts (data-independent
permutations / dtype casts only); P, w, idx ship as bf16.

Per core, each sample's 4096 edges split two ways (v1 was gather-only at
~148 us; v2 measures ~72-87 us depending on machine load):
  - N_G=1280 edges via the v1 ap_gather path: masked per-sample table of
    S^T values (f32 entries = packed bf16 pairs, halving the table to 4MB
    and the bounce+stripe DMA to 512KB; gather idx = j*64 + i//2, the
    i-parity selects the bf16 half post-gather via int32 mask/shift ops
    and two parity-weighted STT accumulations). ~26 ns/index serial per
    GPSIMD core.
  - The other 2816 edges (M_CH=22 chunks of 128) via one-hot matmuls:
    DVE builds bf16 one-hots OH[e,v] = (idx_e == v) with the idx
    broadcast on the MIDDLE axis (inner runs stay step-1; measured
    ~1.4 us per [128,4096], vs ~4.5 us with the broadcast innermost),
    folds w into the I side (~2.6 us), ACT re-lays the weighted I
    one-hot chunk-contiguous (~7.2 us; ACT strided-src copies run ~2x
    the (FD+352)/1.2 formula), then PE accumulates
    W^T_b = sum_s OH_J_s^T @ OHW_I_s into PSUM at ~90 ns per 128-col
    matmul (strided lhsT is free; strided RHS costs 214 ns/MM, both
    strided 522 — hence the ACT re-layout of exactly one operand).
    d_error/d_hw ship transposed so S^T comes out of the S-build and
    pairs directly with W^T in the per-sample <W,S> TT+reduce.

Measured pitfalls baked into the structure (HW, wide-repeat slope timing):
  - 64B-strided SBUF reads cost 2-4x on EVERY engine (DVE 1x mode, ACT
    copies, PE rhs streaming). Strided WRITES are far worse (a DVE op
    writing inner-stride-128 measured ~6x).
  - Only SP + Activation have HWDGE DMA queues (~40 GB/s each); v1's
    single-queue loads serialized ~50 us/iter until split + bf16 diet.
  - Consumer-side DVE ops (gather STT, <W,S> reduces) are DEFERRED one
    iteration (emitted after the next body) so the DVE FIFO never parks
    behind GPSIMD/PE; input loads are emitted one iteration ahead so
    table stripes (which WAR-wait on the previous gather) never block
    them. Gather-only tiles are 2-deep; main tiles 3-deep (NSET=3).
  - TensorTensorReduce and any DVE op with a PSUM operand fault on this
    HW; PSUM is evacuated by ACT before DVE touches it.
  - Engine-balanced components sum to ~45 us but the kernel plateaus at
    ~72-87 us; the residual is cross-engine semaphore/queue latency that
    TimelineSim does not model (its DVE/gather costs are also wrong for
    the fast TT modes). Machine-level wall noise is +/-8 us
    session-to-session; trust only wide-separation (R=257..2049) slopes.
"""

import numpy as np

try:
    from ml_dtypes import bfloat16 as _bf16np
except ImportError:  # pragma: no cover
    _bf16np = np.float32

B, NL, NP, E = 64, 128, 128, 4096
N_CORES = 8
BPC = B // N_CORES  # samples per NeuronCore

# hybrid split: per sample, N_G edges on the GPSIMD gather path, the rest
# on the DVE/PE one-hot path in M_CH chunks of 128 edges
GATHER = True                # hybrid: N_G edges/sample via GPSIMD ap_gather
N_G = 1280 if GATHER else 0
N_FT = 0                     # batches whose weighted-I one-hot is produced
                             # by the 1x DVE fold-transpose; the rest use
                             # fold + ACT re-layout (engine balance)
NSET = 3
M_CH = (E - N_G) // 128
N_DVE = E - N_G              # 2816
QS = 2                       # samples per one-hot batch op
NQ = BPC // QS               # batches
SLOTS = QS * M_CH            # 44 chunk-slots per batch
FDQ = 128 * SLOTS            # one-hot free dim per batch op


def _patch_tile_drain():
    """This toolchain's walrus rejects >1 sem wait on a Drain; split the
    kernel-tail drain into one drain per pending semaphore."""
    import concourse.tile as tile
    from concourse.vector_clock import ScopedClock, VectorClock

    def _drain_and_barrier_split(self, tick_clock, wait_clock):
        nc = self.nc
        gc = tick_clock.global_clock  # VectorClock
        n = len(gc)
        for p in [i for i in range(n) if gc[i] > 0]:
            vec = VectorClock([gc[i] if i == p else 0 for i in range(n)])
            drain_inst = nc.sync.drain()
            wait_clock.add_sem_waits(drain_inst.ins, ScopedClock({None: vec}))
        nc.all_engine_barrier()
        assert self.sems is not None
        popped = nc._tile_sem_poison_stack.pop()
        assert popped is self._sem_poison
        nc.clear_and_free_semaphores(list(self.sems.allocated().values()))
        nc.all_engine_barrier()

    tile.TileContext._drain_and_barrier = _drain_and_barrier_split


def _split_multi_waits(nc, mybir):
    """Walrus codegen accepts at most one sem wait per instruction. Hoist
    extra waits onto preceding same-engine NoOps."""
    k = 0
    for f in nc.m.functions:
        for bb in f.blocks:
            insts = list(bb.instructions)
            out = []
            changed = False
            for ins in insts:
                si = ins.sync_info
                waits = list(si.on_wait) if si is not None and si.on_wait else []
                if len(waits) > 1:
                    changed = True
                    for w in waits[:-1]:
                        nop = mybir.InstNoOp(name=f"xw-{k}", ins=[], outs=[])
                        k += 1
                        nop.engine = ins.engine
                        nop.sync_info = mybir.SyncInfo(on_wait=[w], on_update=[])
                        nc.register_instruction(nop)
                        out.append(nop)
                    ins.sync_info = mybir.SyncInfo(
                        on_wait=[waits[-1]], on_update=list(si.on_update or [])
                    )
                out.append(ins)
            if changed:
                bb.instructions = out


def build_nc(repeat: int = 1, stage: str = "full"):
    """Build the Bass module (single-core SPMD program, run on 8 cores).

    stage in ("loads", "sbuild", "gather", "dve", "full") truncates the
    body for cost bisection.
    """
    import concourse.bass as bass
    import concourse.mybir as mybir
    import concourse.tile as tile
    from concourse import library_config

    _patch_tile_drain()

    AL = mybir.AluOpType
    f32 = mybir.dt.float32
    bf16 = mybir.dt.bfloat16
    i16 = mybir.dt.int16

    nc = bass.Bass(detect_race_conditions=False)

    p_d = nc.dram_tensor("p", [BPC, NL, NP], bf16, kind="ExternalInput")
    if GATHER:
        idxg_d = nc.dram_tensor("idxg", [128, N_G // 16], i16, kind="ExternalInput")
    idxi_d = nc.dram_tensor("idxi", [128, QS * M_CH * NQ], bf16, kind="ExternalInput")
    idxj_d = nc.dram_tensor("idxj", [128, QS * M_CH * NQ], bf16, kind="ExternalInput")
    wdve_d = nc.dram_tensor("wdve", [128, QS * M_CH * NQ], bf16, kind="ExternalInput")
    if GATHER:
        # wg then parity columns (j&1 per gather column), PE-replicated
        wgpar_d = nc.dram_tensor("wgpar", [BPC, 2 * N_G], bf16, kind="ExternalInput")
    # w reshaped so partition 16c+q holds w[c, 256q:256q+256]
    wnat_d = nc.dram_tensor("wnat", [128, 256], bf16, kind="ExternalInput")
    derr_d = nc.dram_tensor("derr", [NP, NP], f32, kind="ExternalInput")
    dhw_d = nc.dram_tensor("dhw", [NP, NP], i32 := mybir.dt.int32, kind="ExternalInput")
    out_d = nc.dram_tensor("out", [1, 1], f32, kind="ExternalOutput")

    # NEFF-embedded constants
    blockones_np = np.zeros((128, BPC), dtype=np.float32)
    for c in range(BPC):
        blockones_np[16 * c:16 * (c + 1), c] = 1.0
    blockones_d = nc.inline_tensor(blockones_np, name="blockones")
    blockonesT_d = nc.inline_tensor(
        np.ascontiguousarray(blockones_np.T).astype(_bf16np), name="blockonesT")
    ones_d = nc.inline_tensor(np.ones((128, 1), dtype=np.float32), name="ones128")
    ident_d = nc.inline_tensor(np.eye(128, dtype=np.float32), name="ident128")  # converted to bf16 tile below
    # iota for one-hot builds: value at free pos (v*SLOTS + s) is v
    iota_np = np.tile(np.repeat(np.arange(128, dtype=np.float32), SLOTS), (128, 1))
    iota_d = nc.inline_tensor(iota_np.astype(_bf16np), name="iotaw")
    msk_d = nc.inline_tensor(np.full((128, 1), -65536, dtype=np.int32), name="hmask")
    sh16_d = nc.inline_tensor(np.full((128, 1), 16, dtype=np.int32), name="sh16")

    with tile.TileContext(nc) as tc:
        with (
            tc.tile_pool(name="persist", bufs=1) as persist,
            tc.tile_pool(name="pp", bufs=2, space="PSUM") as pp,
            tc.tile_pool(name="pt1", bufs=1, space="PSUM") as pt1,
            tc.tile_pool(name="vall", bufs=1, space="PSUM") as vallp,
            tc.tile_pool(name="pred", bufs=1, space="PSUM") as pred,
            tc.tile_pool(name="sdram", bufs=2, space="DRAM") as sdram,
        ):
            if GATHER:
                nc.gpsimd.load_library(library_config.ap_gather)

            # ---- persistent constants
            blockones = persist.tile([128, BPC], f32)
            blockonesT = persist.tile([BPC, 128], bf16)
            ones128 = persist.tile([128, 1], f32)
            ident = persist.tile([128, 128], f32)
            ident16 = persist.tile([128, 128], bf16)
            iota = persist.tile([128, FDQ], bf16)
            nc.sync.dma_start(blockones[:], blockones_d[:])
            nc.sync.dma_start(blockonesT[:], blockonesT_d[:])
            nc.sync.dma_start(ones128[:], ones_d[:])
            nc.sync.dma_start(ident[:], ident_d[:])
            nc.sync.dma_start(iota[:], iota_d[:])

            msk_t = persist.tile([128, 1], i32)
            sh16_t = persist.tile([128, 1], i32)
            nc.sync.dma_start(msk_t[:], msk_d[:])
            nc.sync.dma_start(sh16_t[:], sh16_d[:])
            derr = persist.tile([128, 128], f32)
            dhw = persist.tile([128, 128], i32)
            nc.sync.dma_start(derr[:], derr_d[:])
            nc.sync.dma_start(dhw[:], dhw_d[:])

            # masked gather table (packed bf16 pairs read as f32)
            if GATHER:
                table2 = persist.tile([128, 8192], f32)
            afid = persist.tile([128, 128], bf16)
            relu_e = persist.tile([128, 128], f32)
            mask_e = persist.tile([128, 128], f32)
            if GATHER:
                nc.vector.memset(table2[:, 0:4096], 0.0)
                nc.scalar.memzero(table2[:, 4096:8192])

            nc.vector.tensor_copy(ident16[:], ident[:])

            # A_fid = (dhw == 1) * relu(1 - derr)   (invariant; hoisted)
            nc.scalar.activation(
                relu_e[:], derr[:],
                mybir.ActivationFunctionType.Relu, bias=1.0, scale=-1.0,
            )
            nc.vector.tensor_scalar(
                out=mask_e[:], in0=dhw[:], scalar1=1, scalar2=None,
                op0=AL.is_equal,
            )
            nc.vector.tensor_tensor(
                out=afid[:], in0=relu_e[:], in1=mask_e[:], op=AL.mult,
            )

            # ---- one-hot working buffers (single/double per role)
            ohI_raw = [persist.tile([128, FDQ], bf16, name="ohIraw0")] * 2
            ohIw = [persist.tile([128, FDQ], bf16, name=f"ohIw{k}") for k in range(2)]
            ohJ_raw = [persist.tile([128, FDQ], bf16, name=f"ohJraw{k}")
                       for k in range(2)]
            ohIci = [persist.tile([128, FDQ], bf16, name=f"ohIci{k}") for k in range(2)]
            ttr2_scr = persist.tile([128, 2, 128], bf16, name="ttrscr")
            scr_sh = (persist.tile([128, N_G], bf16, name="scrsh")
                      if GATHER else None)
            # deferred-reduce scratch: written+consumed inside one serial
            # DVE block, so shared across iterations
            two = {"w_sb": [persist.tile([128, BPC, 128], bf16,
                                         name=f"wsbT{k}") for k in range(2)]}
            if GATHER:
                two["wgsrc"] = [persist.tile([BPC, 2 * N_G], bf16,
                                             name=f"wgsT{k}") for k in range(2)]
                two["wgp"] = [persist.tile([128, 2 * N_G], bf16,
                                           name=f"wgpT{k}") for k in range(2)]
                two["val2"] = [persist.tile([128, N_G], f32,
                                            name=f"valT{k}") for k in range(2)]
            sh = {}
            if GATHER:
                sh = {"wpar": persist.tile([128, N_G], bf16, name="wparS"),
                      "wnpar": persist.tile([128, N_G], bf16, name="wnparS"),
                      "vhi": persist.tile([128, N_G], f32, name="vhiS"),
                      "vlo": persist.tile([128, N_G], f32, name="vloS")}

            # ---- per-iteration tiles (double-buffered across repeat iters)
            sets = []
            for s in range(NSET):
                bs = {}
                bs["pall"] = persist.tile([128, BPC, 128], bf16, name=f"pall{s}")
                bs["pt_all"] = persist.tile([128, BPC, 128], bf16, name=f"pt{s}")
                bs["v_sb"] = persist.tile([128, BPC, 128], bf16, name=f"v{s}")
                bs["s_all"] = persist.tile([128, BPC, 128], bf16, name=f"s{s}")
                if GATHER:
                    bs["idxg"] = persist.tile([128, N_G // 16], i16,
                                              name=f"idxg{s}")
                bs["idxi"] = persist.tile([128, NQ * SLOTS], bf16, name=f"idxi{s}")
                bs["idxj"] = persist.tile([128, NQ * SLOTS], bf16, name=f"idxj{s}")
                bs["wdve"] = persist.tile([128, NQ * SLOTS], bf16, name=f"wd{s}")
                bs["w_nat"] = persist.tile([128, 256], bf16, name=f"w{s}")

                bs["wspart"] = persist.tile([128, 1], f32, name=f"wsp{s}")


                
                if GATHER:
                    bs["zacc"] = persist.tile([128, 2], f32, name=f"za{s}")
                bs["ttrcols"] = persist.tile([128, BPC], f32, name=f"ttrc{s}")

                bs["ws8"] = persist.tile([BPC, 1], f32, name=f"ws{s}")
                bs["ysum"] = persist.tile([BPC, 1], f32, name=f"ys{s}")
                bs["zdiv"] = persist.tile([BPC, 1], f32, name=f"zd{s}")
                bs["res"] = persist.tile([1, 1], f32, name=f"res{s}")
                sets.append(bs)

            def body_loads(_it=0):
                bs = sets[_it % NSET]
                # loads run 1 iteration ahead of compute so they are never
                # queued behind table stripes (which wait on the previous
                # gather via the table WAR)
                p_src = bass.AP(
                    tensor=p_d, offset=0,
                    ap=[[128, 128], [NL * NP, BPC], [1, 128]],
                )
                nc.scalar.dma_start(bs["pall"][:], p_src)
                if GATHER:
                    nc.scalar.dma_start(bs["idxg"][:], idxg_d[:])
                nc.scalar.dma_start(bs["idxi"][:], idxi_d[:])
                nc.scalar.dma_start(bs["idxj"][:], idxj_d[:])
                nc.scalar.dma_start(bs["wdve"][:], wdve_d[:])
                nc.sync.dma_start(bs["w_nat"][:], wnat_d[:])
                if GATHER:
                    nc.sync.dma_start(two["wgsrc"][_it % 2][:], wgpar_d[:])

            def body(_it=0):
                bs = sets[_it % NSET]
                pall, pt_all = bs["pall"], bs["pt_all"]
                v_sb, s_all = bs["v_sb"], bs["s_all"]

                nc.vector.tensor_reduce(
                    out=bs["wspart"][:], in_=bs["w_nat"][:],
                    axis=mybir.AxisListType.X, op=AL.add,
                )

                # replicate (wg | parity) [BPC, 2*N_G] to all 16
                # partition rows of each sample via PE (saves 16 row DMAs)
                for ch in (range(0, 2 * N_G, 512) if GATHER else ()):
                    n = min(512, 2 * N_G - ch)
                    rep_ps = pp.tile([128, 512], f32, tag="wgr")
                    nc.tensor.matmul(
                        rep_ps[:, 0:n], lhsT=blockonesT[:],
                        rhs=two["wgsrc"][_it % 2][:, ch:ch + n],
                        start=True, stop=True,
                    )
                    nc.scalar.copy(two["wgp"][_it % 2][:, ch:ch + n], rep_ps[:, 0:n])

                if stage == "loads":
                    nc.vector.memset(bs["res"][:], 0.0)
                    nc.sync.dma_start(out_d[:], bs["res"][:])
                    return

                # ---- S_b = P_b @ A @ P_b^T  (f32, feeds table + TTR)
                for c in range(0, BPC, 2):
                    pt_ps = pt1.tile([128, 2, 128], bf16, tag="ptps")
                    nc.tensor.transpose(pt_ps[:, 0, :], pall[:, c, :], ident16[:])
                    nc.tensor.transpose(pt_ps[:, 1, :], pall[:, c + 1, :], ident16[:])
                    nc.scalar.copy(pt_all[:, c:c + 2, :], pt_ps[:])
                v_ps = vallp.tile([128, BPC, 128], f32, tag="vw")
                nc.tensor.matmul(
                    v_ps[:].rearrange("p a b -> p (a b)")[:, 0:512],
                    lhsT=afid[:],
                    rhs=pt_all[:].rearrange("p a b -> p (a b)")[:, 0:512],
                    start=True, stop=True,
                )
                nc.tensor.matmul(
                    v_ps[:].rearrange("p a b -> p (a b)")[:, 512:1024],
                    lhsT=afid[:],
                    rhs=pt_all[:].rearrange("p a b -> p (a b)")[:, 512:1024],
                    start=True, stop=True,
                )
                nc.scalar.copy(v_sb[:], v_ps[:])

                s_dr = None
                if GATHER:
                    s_dr = sdram.tile([128, BPC, 128], bf16, tag="sdram",
                                      name="sdrt")
                for c in range(0, BPC, 2):
                    s_ps = pp.tile([128, 2, 128], f32, tag="sps")
                    nc.tensor.matmul(
                        s_ps[:, 0, :], lhsT=v_sb[:, c, :], rhs=pt_all[:, c, :],
                        start=True, stop=True,
                    )
                    nc.tensor.matmul(
                        s_ps[:, 1, :], lhsT=v_sb[:, c + 1, :],
                        rhs=pt_all[:, c + 1, :], start=True, stop=True,
                    )
                    nc.scalar.copy(s_all[:, c:c + 2, :], s_ps[:])
                    if GATHER:
                        eng = nc.scalar if c % 4 == 0 else nc.sync
                        eng.dma_start(s_dr[:, c:c + 2, :],
                                      s_all[:, c:c + 2, :])
                # 16 stripe reads: table2[{16c+q}, 1024q:1024q+1024] =
                #   S_c rows [8q, 8q+8)
                for q in (range(16) if GATHER else ()):
                    dst = table2[q::16, 512 * q:512 * (q + 1)]
                    eng = nc.sync if q % 2 == 0 else nc.scalar
                    eng.dma_start(
                        dst,
                        s_dr[:].rearrange("l c x -> c l x")[
                            :, 8 * q:8 * q + 8, :].bitcast(f32),
                    )

                if stage == "sbuild":
                    nc.vector.memset(bs["res"][:], 0.0)
                    nc.sync.dma_start(out_d[:], bs["res"][:])
                    return

                # ---- gather path: one ap_gather (all 8 cores, sample/core)
                if GATHER and stage in ("gather", "full"):
                    nc.gpsimd.ap_gather(
                        out_ap=two["val2"][_it % 2][:].unsqueeze(2),
                        in_ap=table2[:].unsqueeze(2),
                        idxs_ap=bs["idxg"][:],
                        channels=128, num_elems=8192, d=1,
                        num_idxs=N_G,
                    )

                # ---- one-hot scatter path, NQ batches of QS samples.
                # W accumulators reuse the vall PSUM banks (V is fully
                # consumed by the S matmuls above before W writes start).
                if stage in ("dve0", "dve1", "dve2", "dve", "full"):
                    wps4 = vallp.tile([128, BPC, 128], f32, tag="vw")
                    for qb in range(NQ):
                        sl = slice(qb * SLOTS, (qb + 1) * SLOTS)
                        idxI = bs["idxi"][:, sl].unsqueeze(1).broadcast_to(
                            [128, 128, SLOTS])
                        idxJ = bs["idxj"][:, sl].unsqueeze(1).broadcast_to(
                            [128, 128, SLOTS])
                        wq = bs["wdve"][:, sl].unsqueeze(1).broadcast_to(
                            [128, 128, SLOTS])
                        i3 = iota[:].rearrange("p (v s) -> p v s", v=128, s=SLOTS)
                        oJ = ohJ_raw[qb % 2][:].rearrange(
                            "p (v s) -> p v s", v=128, s=SLOTS)
                        oI = ohI_raw[qb % 2][:].rearrange(
                            "p (v s) -> p v s", v=128, s=SLOTS)
                        oIw = ohIw[qb % 2][:].rearrange(
                            "p (v s) -> p v s", v=128, s=SLOTS)
                        oIci = ohIci[qb % 2][:].rearrange(
                            "p (s v) -> p s v", s=SLOTS, v=128)
                        # J one-hot stays [p, v, s]: strided lhsT is free
                        nc.vector.tensor_tensor(
                            out=oJ, in0=idxJ, in1=i3, op=AL.is_equal)
                        # I one-hot weighted, must end [p, s, v]-contiguous
                        # (rhs). Two routes, split for engine balance.
                        nc.vector.tensor_tensor(
                            out=oI, in0=idxI, in1=i3, op=AL.is_equal)
                        if qb < N_FT:
                            # 1x DVE fold-transpose straight into [s, v]
                            # (iterate in the contiguous OUT order; the
                            # strided reads cost 1x, strided writes would
                            # cost far more)
                            wq_t = bs["wdve"][:, sl].unsqueeze(2).broadcast_to(
                                [128, SLOTS, 128])
                            nc.vector.tensor_tensor(
                                out=oIci, in0=wq_t,
                                in1=oI.transpose([0, 2, 1]), op=AL.mult)
                        else:
                            # fast 2x fold in [v, s], then ACT re-layout
                            nc.vector.tensor_tensor(
                                out=oIw, in0=wq, in1=oI, op=AL.mult)
                            if stage != "dve0":
                                nc.scalar.copy(
                                    oIci, oIw.transpose([0, 2, 1]))
                        # PE: accumulate W_b for the QS samples of this batch
                        if stage in ("dve0", "dve1"):
                            continue
                        for bl in range(QS):
                            b = qb * QS + bl
                            for c in range(M_CH):
                                s = bl * M_CH + c
                                lhs = ohJ_raw[qb % 2][:].rearrange(
                                    "p (v s) -> p v s", v=128, s=SLOTS)[:, :, s]
                                rhs = ohIci[qb % 2][:].rearrange(
                                    "p (s v) -> p s v", s=SLOTS, v=128)[:, s, :]
                                nc.tensor.matmul(
                                    wps4[:, b, :], lhsT=lhs, rhs=rhs,
                                    start=(c == 0), stop=(c == M_CH - 1),
                                )
                        # ACT evacuates W PSUM->SBUF per 4-sample group;
                        # the <W_b, S_b> DVE reduction is deferred to
                        # body_tail so it never stalls the next iteration's
                        # one-hot builds in the DVE FIFO
                        if stage == "dve2":
                            continue
                        if (qb * QS) % 4 == 4 - QS:
                            g0 = (qb + 1) * QS - 4
                            nc.scalar.copy(
                                two["w_sb"][_it % 2][:, g0:g0 + 4, :],
                                wps4[:, g0:g0 + 4, :])
                if stage in ("dve0", "dve1", "dve2", "gather"):
                    nc.vector.memset(bs["ttrcols"][:], 0.0)

                if GATHER and stage not in ("gather", "full"):
                    nc.vector.memset(bs["zacc"][:], 0.0)

            def body_tail(_it=0):
                # deferred finals for iteration _it (emitted after iteration
                # _it+1's matmuls so PE queue isn't head-of-line-blocked)
                bs = sets[_it % NSET]
                # gather-path weighted reduction, deferred so iteration
                # _it+1's DVE one-hots are not queued behind the wait on
                # gather(_it) (that queue stall serialized gather vs the
                # whole one-hot path: 71 -> ~40 us/iter)
                if GATHER and stage in ("gather", "full"):
                    # packed-pair table: each gathered f32 is (lo, hi) bf16
                    # pair; parity of j selects. hi = bits&0xFFFF0000,
                    # lo = bits<<16, read as f32 (= bf16 value).
                    v_i32 = two["val2"][_it % 2][:].bitcast(mybir.dt.int32)
                    nc.vector.tensor_scalar(
                        out=sh["vhi"][:].bitcast(mybir.dt.int32), in0=v_i32,
                        scalar1=msk_t[:], scalar2=None,
                        op0=AL.bitwise_and,
                    )
                    nc.vector.tensor_scalar(
                        out=sh["vlo"][:].bitcast(mybir.dt.int32), in0=v_i32,
                        scalar1=sh16_t[:], scalar2=None,
                        op0=AL.logical_shift_left,
                    )
                    # wpar = wg*par, wnpar = wg - wpar
                    nc.vector.tensor_tensor(
                        out=sh["wpar"][:], in0=two["wgp"][_it % 2][:, 0:N_G],
                        in1=two["wgp"][_it % 2][:, N_G:2 * N_G], op=AL.mult,
                    )
                    nc.vector.tensor_tensor(
                        out=sh["wnpar"][:], in0=two["wgp"][_it % 2][:, 0:N_G],
                        in1=sh["wpar"][:], op=AL.subtract,
                    )
                    nc.vector.scalar_tensor_tensor(
                        out=scr_sh[:],
                        in0=sh["vhi"][:],
                        scalar=0.0, in1=sh["wpar"][:],
                        op0=AL.add, op1=AL.mult,
                        accum_out=bs["zacc"][:, 0:1],
                    )
                    nc.vector.scalar_tensor_tensor(
                        out=scr_sh[:],
                        in0=sh["vlo"][:],
                        scalar=0.0, in1=sh["wnpar"][:],
                        op0=AL.add, op1=AL.mult,
                        accum_out=bs["zacc"][:, 1:2],
                    )
                if stage in ("dve", "full"):
                    for b2 in range(0, BPC, 2):
                        nc.vector.tensor_tensor(
                            out=ttr2_scr[:],
                            in0=two["w_sb"][_it % 2][:, b2:b2 + 2, :],
                            in1=bs["s_all"][:, b2:b2 + 2, :],
                            op=AL.mult,
                        )
                        nc.vector.tensor_reduce(
                            out=bs["ttrcols"][:, b2:b2 + 2],
                            in_=ttr2_scr[:],
                            axis=mybir.AxisListType.X, op=AL.add,
                        )
                yv_ps = pred.tile([BPC, 4], f32, tag="yv")
                # y = gather-part (hi+lo) + dve-part, accumulated in PSUM
                if GATHER:
                    nc.tensor.matmul(
                        yv_ps[:, 0:1], lhsT=blockones[:],
                        rhs=bs["zacc"][:, 0:1], start=True, stop=False,
                    )
                    nc.tensor.matmul(
                        yv_ps[:, 0:1], lhsT=blockones[:],
                        rhs=bs["zacc"][:, 1:2], start=False, stop=False,
                    )
                nc.tensor.matmul(
                    yv_ps[:, 0:1], lhsT=bs["ttrcols"][:], rhs=ones128[:],
                    start=not GATHER, stop=True,
                )
                nc.tensor.matmul(
                    yv_ps[:, 2:3], lhsT=blockones[:], rhs=bs["wspart"][:],
                    start=True, stop=True,
                )
                nc.vector.tensor_scalar(
                    out=bs["ws8"][:], in0=yv_ps[:, 2:3], scalar1=1e-8,
                    scalar2=None, op0=AL.max,
                )
                nc.vector.reciprocal(bs["ws8"][:], bs["ws8"][:])
                nc.vector.tensor_tensor(
                    out=bs["zdiv"][:], in0=yv_ps[:, 0:1], in1=bs["ws8"][:],
                    op=AL.mult,
                )
                zz_ps = yv_ps[0:1, 3:4]
                nc.tensor.matmul(
                    zz_ps, lhsT=bs["zdiv"][:], rhs=ones128[0:BPC, :],
                    start=True, stop=True,
                )
                nc.vector.tensor_copy(bs["res"][:], zz_ps)
                nc.vector.tensor_scalar_mul(bs["res"][:], bs["res"][:],
                                            -1.0 / B)
                nc.sync.dma_start(out_d[:], bs["res"][:])

            full_tail = stage in ("full", "dve", "dve0", "dve1", "dve2", "gather")

            def emit_seq(n):
                # loads(it+1) are emitted before compute(it) so the input
                # DMAs of the next iteration precede this iteration's
                # stripes in the queue FIFOs
                body_loads(0)
                for it in range(n):
                    if it + 1 < n:
                        body_loads(it + 1)
                    body(it)
                    if full_tail and it > 0:
                        body_tail(it - 1)
                if full_tail:
                    body_tail(n - 1)

            if repeat == 1:
                emit_seq(1)
            elif repeat <= 8:
                emit_seq(repeat)
            else:
                UN = 8
                with tc.For_i(0, repeat // UN, 1):
                    emit_seq(UN)
                if repeat % UN:
                    emit_seq(repeat % UN)

    _split_multi_waits(nc, mybir)
    mybir.codegen_inst_isa_subclasses(nc)
    return nc


def _shard_inputs(P, d_error, edge_weights, d_hw, edge_pairs):
    ep = np.asarray(edge_pairs)
    i_all = ep[..., 0].astype(np.int16)   # [B, E]
    j_all = ep[..., 1].astype(np.int16)
    w_all = np.asarray(edge_weights, dtype=np.float32)
    # ship A_fid inputs TRANSPOSED: the device then builds S^T = P A^T P^T,
    # which is the layout the W^T accumulation and the gather table want
    derr = np.ascontiguousarray(np.asarray(d_error, dtype=np.float32).T)
    dhw = np.ascontiguousarray(np.asarray(d_hw, dtype=np.int32).T)

    # --- gather part: edge(q, t) = 256q + t for t < N_G//16; gather column
    # k (of N_G) on partition-row q = k // (N_G//16)... layout: partition
    # 16c+q holds sample c's indices for edges {256q + t : t in [0, TG)}.
    TG = N_G // 16
    t_idx = np.arange(TG)
    q_idx = np.arange(16)
    gather_edges = (256 * q_idx[:, None] + t_idx[None, :])  # [16, TG]

    # --- dve part: leftover lin index L in [0, N_DVE): q = L // TL,
    # t = L % TL, edge = 256q + TG + t ; slot mapping L = c*128 + p
    TL = 256 - TG
    Lp = np.arange(N_DVE)
    dve_edges = 256 * (Lp // TL) + TG + (Lp % TL)   # [N_DVE] in L order
    # tile [128 p, M_CH c] per sample: value at (p, c) = edge L = c*128 + p
    dve_map = dve_edges.reshape(M_CH, 128).T         # [128, M_CH]

    # gather output column k (of N_G, same on all 16 partitions of a
    # core) uses the index stored at wrapped position (row k%16, col
    # k//16), i.e. edge 256*(k%16) + k//16 with k//16 < TG. w_g pairs w
    # with those columns, replicated to all 16 partition rows of a sample
    # (masked table zeros the 15 non-owner rows) -- v1-validated layout.
    k_idx = np.arange(N_G)
    gcol_edges = 256 * (k_idx % 16) + (k_idx // 16)  # [N_G], t = k//16 < TG

    in_maps = []
    for core in range(N_CORES):
        sl = slice(BPC * core, BPC * (core + 1))
        i_c = i_all[sl]   # [BPC, E] int16
        j_c = j_all[sl]
        w_c = w_all[sl]

        if GATHER:
            # idxg [128, TG]: partition 16c+q, position t -> packed-pair
            # index of sample c, edge 256q + t (bf16 pairs: i*64 + j//2)
            idxg = np.zeros((128, TG), dtype=np.int16)
            for c in range(BPC):
                for q in range(16):
                    e_ids = gather_edges[q]
                    idxg[16 * c + q, :] = (
                        j_c[c, e_ids].astype(np.int32) * 64
                        + (i_c[c, e_ids].astype(np.int32) >> 1)
                    ).astype(np.int16)

        # dve tiles [128, NQ*SLOTS]: slot s = b*M_CH + c global order:
        # columns grouped per batch qb: for qb, local (bl, c):
        # col = qb*SLOTS + bl*M_CH + c, sample b = qb*QS+bl, edge =
        # dve_map[p, c]
        idxi = np.zeros((128, NQ * SLOTS), dtype=np.float32)
        idxj = np.zeros((128, NQ * SLOTS), dtype=np.float32)
        wdve = np.zeros((128, NQ * SLOTS), dtype=np.float32)
        for b in range(BPC):
            qb, bl = divmod(b, QS)
            cols = qb * SLOTS + bl * M_CH + np.arange(M_CH)
            idxi[:, cols] = i_c[b, dve_map]
            idxj[:, cols] = j_c[b, dve_map]
            wdve[:, cols] = w_c[b, dve_map]

        if GATHER:
            # (wg | j-parity) per gather column, PE-replicated on device
            wgpar = np.concatenate(
                [w_c[:, gcol_edges],
                 (i_c[:, gcol_edges] & 1).astype(np.float32)], axis=1)

        im = {
            "p": np.ascontiguousarray(
                np.asarray(P[sl], dtype=np.float32).astype(_bf16np)),
            "idxi": idxi.astype(_bf16np),
            "idxj": idxj.astype(_bf16np),
            "wdve": wdve.astype(_bf16np),
            "wnat": np.ascontiguousarray(
                w_c.reshape(BPC * 16, 256)).astype(_bf16np),
            "derr": derr,
            "dhw": dhw,
        }
        if GATHER:
            im["idxg"] = idxg
            im["wgpar"] = np.ascontiguousarray(wgpar).astype(_bf16np)
        in_maps.append(im)
    return in_maps


def kernel(P, d_error, edge_weights, d_hw, edge_pairs):
    from concourse.bass_utils import run_bass_kernel_spmd

    nc = build_nc()
    in_maps = _shard_inputs(P, d_error, edge_weights, d_hw, edge_pairs)
    res = run_bass_kernel_spmd(nc, in_maps, core_ids=list(range(N_CORES)))
    total = np.float32(0.0)
    for r in res.results:
        total += np.float32(r["out"][0, 0])
    return np.float32(total)
